# revision 53
# baseline (speedup 1.0000x reference)
"""DMRG two-site effective Hamiltonian application (ApplyMPO) on 8 trn2 cores.

Math (reference):
  res[h,i,j,k] = sum_{a,b,c,d,e,f,g} L[b,h,a] M1[b,d,i,c] M2[d,f,j,e]
                                     R[f,k,g] psi[a,c,e,g]

Device algorithm (per core, output bond h sharded 8 x 128), all bf16 with
fp32 PSUM accumulation:
  Q[(b,c,e),(i,j,f)] = sum_d M1[b,d,i,c] M2[d,f,j,e]            (host, 400 els)
  step1: T1[h; pack, (g6,bce)] = sum_a L[b,h,a] psi[a,(c,e),g]  (PE, K=a)
         written in 128-col packs: col = g6*20 + (b*4+ce), 6 g's per pack
  flipA: T1P[(g6,bce); pack, h] = DMA XBAR transpose of T1       (DMA engine)
  mix:   T3H[h; (i,j,f), g]     = T1P[pack]^T @ Q6P              (PE, K=20 eff)
  flipB: T3G[g; (ijf,blk), h]   = PE transpose of T3H g-slices   (PE, 1 c/r)
  step4: res[h; ij,k]          += T3G[ijf,blk]^T @ R^T[f][g,k]   (PE, K=g, acc f)
flipA rides the DMA XBAR (256B packets, ~120GB/s aggregate) whose latency is
hidden behind interleaved step-4 work; flipB stays on the PE because a second
XBAR stream saturates the DMA fabric and stalls the mix.
"""

import numpy as np
import ml_dtypes

import concourse.bacc as bacc
import concourse.mybir as mybir
import concourse.tile as tile
from concourse import bass_utils

F32 = mybir.dt.float32
BF16 = mybir.dt.bfloat16
BF_NP = ml_dtypes.bfloat16

CHI = 1024
W = 5
D = 2
NCORES = 8
H = CHI // NCORES  # 128, h rows per core
NPACK = 43  # 6-g packs per 256-g quarter: 42 full + one 4-g tail

_nc_cache = None


def _build_nc():
    nc = bacc.Bacc("TRN2", target_bir_lowering=False)
    # host-prearranged: psi[ac, q, a_lo, ce, g256]; lt[b, a_lo, ac, h]; rt[blk, g_lo, f, k]
    psi = nc.dram_tensor("psi", [8, 4, 128, 4, 256], BF16, kind="ExternalInput")
    lt = nc.dram_tensor("lt", [5, 128, 8, H], BF16, kind="ExternalInput")
    rt = nc.dram_tensor("rt", [8, 128, 5, 1024], BF16, kind="ExternalInput")
    q6 = nc.dram_tensor("q6", [120, 128], BF16, kind="ExternalInput")
    q4 = nc.dram_tensor("q4", [80, 128], BF16, kind="ExternalInput")
    idn = nc.dram_tensor("idn", [128, 128], BF16, kind="ExternalInput")
    res = nc.dram_tensor("res", [H, 4096], F32, kind="ExternalOutput")  # h;(i,j,k)

    with tile.TileContext(nc) as tc:
        with (
            tc.tile_pool(name="const", bufs=1) as const_pool,
            tc.tile_pool(name="psis", bufs=2) as psi_pool,
            tc.tile_pool(name="t1", bufs=2) as t1_pool,
            tc.tile_pool(name="t1p", bufs=2) as t1p_pool,
            tc.tile_pool(name="t3h", bufs=2) as t3h_pool,
            tc.tile_pool(name="t3g", bufs=2) as t3g_pool,
            tc.tile_pool(name="rblk", bufs=2) as rblk_pool,
            tc.tile_pool(name="resp", bufs=1) as res_pool,
            tc.tile_pool(name="ps_s1", bufs=2, space="PSUM") as ps_s1,
            tc.tile_pool(name="ps_mid", bufs=2, space="PSUM") as ps_mid,
            tc.tile_pool(name="ps_s4", bufs=2, space="PSUM") as ps_s4,
        ):
            # ---- static loads (only b=0 weights gate the first matmul) ----
            lt_sb = const_pool.tile([128, 5, 8, H], BF16)  # [a_lo; b, ac, h]
            lt_r = lt.ap().rearrange("b p ac h -> p b ac h")
            # weights go on the Activation HWDGE queue so they don't serialize
            # behind the psi tiles on the SP queue; the b=0/ac=0 chunk gates
            # the very first matmul, so load it alone first
            nc.scalar.dma_start(lt_sb[:, 0, 0], lt_r[:, 0, 0])
            nc.scalar.dma_start(lt_sb[:, 0, 1:8], lt_r[:, 0, 1:8])
            q6_sb = const_pool.tile([120, 128], BF16)
            q4_sb = const_pool.tile([80, 128], BF16)
            idn_sb = const_pool.tile([128, 128], BF16)
            res_sb = res_pool.tile([128, 4096], F32)

            def load_rest_of_consts():
                # on the sync queue: the scalar queue's XBAR transpose for
                # flipA(q0) must not queue behind these transfers
                for b in range(1, 5):
                    nc.sync.dma_start(lt_sb[:, b], lt_r[:, b])
                nc.sync.dma_start(q6_sb[:], q6.ap())
                nc.sync.dma_start(q4_sb[:], q4.ap())
                nc.sync.dma_start(idn_sb[:], idn.ap())

            evac_ct = 0

            def evac_copy(out, in_):
                # DVE is ~2x faster than ACT for copies; give ACT 1 in 3.
                nonlocal evac_ct
                evac_ct += 1
                if evac_ct % 3 == 0:
                    nc.scalar.copy(out, in_)
                else:
                    nc.vector.tensor_copy(out, in_)

            # deferred step-4 work: list of closures (one per psum group)
            pending_s4 = []

            def fire_s4(n=1):
                for _ in range(n):
                    if pending_s4:
                        pending_s4.pop(0)()

            def load_psi(qq):
                # one [a_lo; ac, ce, g] tile per quarter: a single DMA and a
                # single completion semaphore instead of 8 trickling ones
                pt = psi_pool.tile([128, 8, 4, 256], BF16, tag="psi")
                src_q = psi.ap()[:, qq].rearrange("ac p c g -> p ac c g")
                if qq == 0:
                    # trickle per-ac so step1(q0) starts as data arrives; the
                    # first matmul gates on the ac=0 / ce 0:2 slice only
                    nc.sync.dma_start(pt[:, 0, 0:2], psi.ap()[0, 0, :, 0:2])
                    nc.sync.dma_start(pt[:, 0, 2:4], psi.ap()[0, 0, :, 2:4])
                    for ac in range(1, 8):
                        nc.sync.dma_start(pt[:, ac], src_q[:, ac])
                else:
                    nc.sync.dma_start(pt[:], src_q)
                return pt

            def alloc_t1q():
                t1q = t1_pool.tile([128, NPACK, 128], BF16, tag="t1q")
                # packed destination views: col = g6*20 + (b*4 + ce)
                dstA = t1q[:, 0:42, 0:120].rearrange("p n (g c) -> p n g c", c=20)
                dstB = t1q[:, 42, 0:80].rearrange("p (g c) -> p g c", c=20)
                return {"t1q": t1q, "dstA": dstA, "dstB": dstB}

            def step1_b(st, psis, b):
                ps1 = ps_s1.tile([128, 4, 256], F32, tag="s1")  # 2 banks
                ps1_flat = ps1[:].rearrange("p c g -> p (c g)")
                for ac in range(8):
                    lhsT = lt_sb[:, b, ac]
                    psi_flat = psis[:, ac].rearrange("p c g -> p (c g)")
                    for cep in range(2):  # one 512-wide MM per PSUM bank
                        nc.tensor.matmul(
                            ps1_flat[:, cep * 512:(cep + 1) * 512],
                            lhsT,
                            psi_flat[:, cep * 512:(cep + 1) * 512],
                            start=(ac == 0),
                            stop=(ac == 7),
                        )
                nc.vector.tensor_copy(
                    st["dstA"][:, :, :, b * 4:b * 4 + 2],
                    ps1[:, 0:2, 0:252].rearrange("p c (n g) -> p n g c", g=6),
                )
                nc.scalar.copy(
                    st["dstA"][:, :, :, b * 4 + 2:b * 4 + 4],
                    ps1[:, 2:4, 0:252].rearrange("p c (n g) -> p n g c", g=6),
                )
                nc.vector.tensor_copy(
                    st["dstB"][:, :, b * 4:b * 4 + 2],
                    ps1[:, 0:2, 252:256].rearrange("p c g -> p g c"),
                )
                nc.scalar.copy(
                    st["dstB"][:, :, b * 4 + 2:b * 4 + 4],
                    ps1[:, 2:4, 252:256].rearrange("p c g -> p g c"),
                )

            st_by_q = {}
            psis_by_q = {}
            for q in range(4):  # g-quarters
                # ---------- step 1: T1q[h; pack, (g6, bce)] ----------
                if q == 0:
                    psis_by_q[0] = load_psi(0)
                    st_by_q[0] = alloc_t1q()
                    load_rest_of_consts()
                if q < 3:
                    psis_by_q[q + 1] = load_psi(q + 1)
                # prefetch this quarter's R blocks early (used by step4(q),
                # which fires during q+1's body or, for q=3, during flipB)
                rbt = rblk_pool.tile([128, 2, 5, 1024], BF16, tag="rblk")
                nc.sync.dma_start(
                    rbt[:], rt.ap()[q * 2:q * 2 + 2].rearrange("b p f k -> p b f k")
                )
                rbs = rbt
                # leading b-blocks of this quarter were hoisted into the
                # previous body (3 blocks into q0's flipA window, else 2)
                for b in range(0 if q == 0 else (3 if q == 1 else 2), 5):
                    step1_b(st_by_q[q], psis_by_q[q], b)
                    if b in (2, 4):
                        fire_s4(1)  # step4(q-1) groups during step1(q)

                # ---------- flipA: DMA XBAR transpose to T1P[(g6,bce); pack, h]
                t1q_flat = st_by_q[q]["t1q"][:].rearrange("p n c -> p (n c)")
                t1p = t1p_pool.tile([128, NPACK, 128], BF16, tag="t1p")
                with tc.high_priority():
                    nc.scalar.dma_start(
                        t1p[:, 0:21, :], t1q_flat[:, 0:21 * 128], transpose=True
                    )
                    (nc.scalar if q == 1 else nc.sync).dma_start(
                        t1p[:, 21:NPACK, :], t1q_flat[:, 21 * 128:NPACK * 128],
                        transpose=True,
                    )
                # hoist the next quarter's first two step-1 blocks here: they
                # keep the PE busy while the flipA XBAR transpose lands
                if q < 3:
                    st_by_q[q + 1] = alloc_t1q()
                    step1_b(st_by_q[q + 1], psis_by_q[q + 1], 0)
                    fire_s4(1)
                    step1_b(st_by_q[q + 1], psis_by_q[q + 1], 1)
                    fire_s4(1)
                    if q == 0:  # q0 has no deferred step-4 filler
                        step1_b(st_by_q[1], psis_by_q[1], 2)
                else:
                    fire_s4(2)
                fire_s4(1)

                # ---------- mix: T3H[h; ijf20, g256] ----------
                t3h = t3h_pool.tile([128, 20, 256], BF16, tag="t3h")
                groups = [list(range(g0, min(g0 + 4, NPACK))) for g0 in range(0, NPACK, 4)]
                for gi, grp in enumerate(groups):
                    pmq = ps_mid.tile([128, 4, 128], F32, tag="mid")  # 1 bank
                    for k, pack in enumerate(grp):
                        if pack < 42:
                            nc.tensor.matmul(
                                pmq[:, k, :], t1p[0:120, pack, :], q6_sb[:],
                                start=True, stop=True,
                            )
                        else:
                            nc.tensor.matmul(
                                pmq[:, k, :], t1p[0:80, 42, :], q4_sb[:],
                                start=True, stop=True,
                            )
                    nfull = sum(1 for p_ in grp if p_ < 42)
                    if nfull:
                        evac_copy(
                            t3h[:, :, grp[0] * 6:grp[0] * 6 + nfull * 6].rearrange(
                                "p i (k g) -> p k i g", g=6
                            ),
                            pmq[:, 0:nfull, 0:120].rearrange(
                                "p k (i g) -> p k i g", g=6
                            ),
                        )
                    if grp[-1] == 42:
                        evac_copy(
                            t3h[:, :, 252:256],
                            pmq[:, nfull, 0:80].rearrange("p (i g) -> p i g", g=4),
                        )
                    if gi in (2, 5):
                        fire_s4(1)
                fire_s4(1)

                # ---------- step 4: defer per-(ij,kh) groups into q+1's body
                def make_s4(qq, t3g_, rbs_):
                    def emit(ij, kh):
                        ps4 = ps_s4.tile([128, 512], F32, tag="s4")  # 1 bank
                        for blk2 in range(2):
                            for f in range(5):
                                nc.tensor.matmul(
                                    ps4[:],
                                    t3g_[:, (ij * 5 + f) * 2 + blk2, :],
                                    rbs_[:, blk2, f, kh * 512:(kh + 1) * 512],
                                    start=(blk2 == 0 and f == 0),
                                    stop=(blk2 == 1 and f == 4),
                                )
                        dst = res_sb[:, ij * 1024 + kh * 512:ij * 1024 + kh * 512 + 512]
                        if qq == 0:
                            evac_copy(dst, ps4[:])
                        else:
                            nc.vector.tensor_add(dst, dst, ps4[:])
                        if qq == 3:
                            nc.sync.dma_start(
                                res.ap()[:, ij * 1024 + kh * 512:ij * 1024 + kh * 512 + 512],
                                dst,
                            )

                    return [
                        (lambda ij=ij, kh=kh: emit(ij, kh))
                        for ij in range(4)
                        for kh in range(2)
                    ]

                # ---------- flipB: PE transpose to T3G[g; (ijf, blk), h]
                t3g = t3g_pool.tile([128, 40, 128], BF16, tag="t3g")
                if q == 3:
                    # last quarter: its step4 has no later phase to hide in,
                    # so interleave it into flipB as soon as data is ready
                    pending_s4.extend(make_s4(q, t3g, rbs))
                for ng in range(10):  # 4 transposes per PSUM bank
                    pb = ps_mid.tile([128, 4, 128], BF16, tag="mid")
                    for j in range(4):
                        n = ng * 4 + j  # n = ijf*2 + blk2
                        nc.tensor.transpose(
                            pb[:, j, :],
                            t3h[:, n // 2, (n % 2) * 128:(n % 2) * 128 + 128],
                            idn_sb[:],
                        )
                    evac_copy(
                        t3g[:, ng * 4:(ng + 1) * 4, :].rearrange("p n h -> p (n h)"),
                        pb[:].rearrange("p j h -> p (j h)"),
                    )
                    if q == 3 and ng in (3, 6):
                        fire_s4(2)  # ij0 after n<=15 done, ij1 after n<=27
                if q < 3:
                    pending_s4.extend(make_s4(q, t3g, rbs))

            # flush remaining deferred step-4 work (last quarter)
            fire_s4(len(pending_s4))
    nc.compile()
    return nc


def _host_inputs(psi_flat, L, M1, M2, R):
    # psi[a,ce,g] -> [ac, q, a_lo, ce, g256]
    psi = np.ascontiguousarray(
        psi_flat.reshape(8, 128, 4, 4, 256).transpose(0, 3, 1, 2, 4)
    ).astype(BF_NP)
    # R[f,k,g] -> RT[f,g,k] -> [blk, g_lo, f, k]
    RT = np.ascontiguousarray(
        R.transpose(2, 0, 1).reshape(8, 128, 5, 1024)
    ).astype(BF_NP)
    Q = np.einsum("bdic,dfje->bceijf", M1, M2).reshape(20, 20).astype(np.float32)
    rows = np.arange(20)
    Q6P = np.zeros((120, 128), np.float32)
    for g6 in range(6):
        Q6P[np.ix_(g6 * 20 + rows, rows * 6 + g6)] = Q
    Q4P = np.zeros((80, 128), np.float32)
    for g4 in range(4):
        Q4P[np.ix_(g4 * 20 + rows, rows * 4 + g4)] = Q
    Q6P = Q6P.astype(BF_NP)
    Q4P = Q4P.astype(BF_NP)
    idn = np.eye(128, dtype=np.float32).astype(BF_NP)
    in_maps = []
    for c in range(NCORES):
        LT = np.ascontiguousarray(
            L[:, c * H:(c + 1) * H, :].transpose(0, 2, 1).reshape(5, 8, 128, H)
            .transpose(0, 2, 1, 3)
        ).astype(BF_NP)  # [b, a_lo, ac, h]
        in_maps.append({"psi": psi, "lt": LT, "rt": RT, "q6": Q6P, "q4": Q4P, "idn": idn})
    return in_maps


def kernel(**inputs):
    psi_flat = np.asarray(inputs["psi_flat"], np.float32)
    L = np.asarray(inputs["L"], np.float32)
    M1 = np.asarray(inputs["M1"], np.float32)
    M2 = np.asarray(inputs["M2"], np.float32)
    R = np.asarray(inputs["R"], np.float32)

    global _nc_cache
    if _nc_cache is None:
        _nc_cache = _build_nc()
    nc = _nc_cache

    in_maps = _host_inputs(psi_flat, L, M1, M2, R)
    out = bass_utils.run_bass_kernel_spmd(nc, in_maps, core_ids=list(range(NCORES)))
    parts = [out.results[c]["res"] for c in range(NCORES)]
    return np.concatenate(parts, axis=0).reshape(-1)
